# revision 18
# baseline (speedup 1.0000x reference)
"""Per-patch dynamic conv (nn_DynaMicConv) as a Bass/Tile kernel on 8 TRN2 cores.

Math: for each patch p of a 14x14 grid over a 224x224 image, out[b, :, p] =
W[p] @ patch_pixels[b, p] + bias[p], i.e. 196 independent [64,768] x [768,768]
matmuls.

Design (measured-driven):
  * W rides as fp8 E3M4 (TRN float8e3, 4 mantissa bits): half the W bytes of
    f16 at ~1.33e-2 rel err (gate 2e-2). A global power-of-2 scale (W*128
    fits e3m4's +-15.5 range) is folded into x (x/128, exact in f16). Per-
    core load traffic: W 14.75MB fp8 + x 2.46MB f16 + bias 1.28MB f16.
  * The PE is the pacer at this traffic: the moving operand streams 1 col/
    cycle regardless of dtype, and col-tiled halves do NOT overlap (K=128
    LDWEIGHTS conflicts on every row group), so wall time ~= streamed cols.
    Patches run in PAIRS via column tiling (patch 2j -> PSUM partitions
    0:64, patch 2j+1 -> 64:128) which at least halves PSUM/eviction count,
    and the bias matmuls (14% of streamed cols) are GONE: bias arrives
    host-replicated [64 rows, 768] per patch in f16 and is added by the DVE
    during PSUM->SBUF eviction (tensor_add).
  * x and replicated bias load resident up front; W groups alternate the
    two HWDGE rings; stores ride SWDGE (gpsimd) in ~3-pair segments -- a
    mid-ring HWDGE store blocks W loads behind it (FIFO), and SWDGE costs
    ~5us serialized per dma_start, so few segments + deep output buffers
    (no store->eviction backpressure).

Sharding: patch-parallel, P=196 padded to 200, 25 per core (12 pairs + 1).
Output DRAM per core is [128, 13*768] f16: col block j holds pair j (rows
0:64 = patch 2j, rows 64:128 = patch 2j+1); block 12 rows 0:64 = patch 24.
"""

import numpy as np

import concourse.bacc as bacc
import concourse.mybir as mybir
import concourse.tile as tile
from concourse.bass_utils import run_bass_kernel_spmd

B, CIN, IMG, PS, G = 64, 3, 224, 16, 14
P = G * G                 # 196 patches
COUT = 768
K = CIN * PS * PS         # 768 contraction
KCH = K // 128            # 6 k-chunks
NCORES = 8
PPC = (P + NCORES - 1) // NCORES   # 25 patches per core (padded)
PPAD = PPC * NCORES                # 200
NPAIR = PPC // 2                   # 12 full pairs; local patch 24 runs alone
NBLK = NPAIR + 1                   # output col blocks

# W DMA groups in compute order: (patch list, HWDGE ring)
WGROUPS = [
    ([0, 1, 2, 3], "sync"),
    ([4, 5, 6, 7, 8, 9], "scalar"),
    ([10, 11, 12, 13, 14, 15], "sync"),
    ([16, 17, 18, 19, 20, 21], "scalar"),
    ([22, 23], "sync"),
    ([24], "scalar"),
]
# store segments in pair blocks [lo, hi) -- all on SWDGE
STORE_CUTS = [0, 2, 5, 8, 11, 12]

F32 = mybir.dt.float32
F16 = mybir.dt.float16
F8 = mybir.dt.float8e3   # TRN E3M4

WSCALE = 128.0           # power of 2: folded into x exactly
E3M4_MAX = 15.5

WBUFS = 3

# test.py hooks
TRACE = False
TRACE_CORES = [0]
LAST_RESULT = None

_CACHE = {}

WB = KCH * COUT          # W bytes per patch per partition (fp8)


def _build():
    nc = bacc.Bacc("TRN2", target_bir_lowering=False, debug=False)
    sizes = sorted({len(pl) for pl, _ in WGROUPS})
    cnt = {s: sum(1 for pl, _ in WGROUPS if len(pl) == s) for s in sizes}
    w_d = {s: nc.dram_tensor(f"w{s}", [cnt[s], 128, s * WB], F8,
                             kind="ExternalInput") for s in sizes}
    XSPL = 10   # x arrives in two chunks so the first W group isn't gated
    x0_d = nc.dram_tensor("x0", [128, XSPL * KCH * B], F16,
                          kind="ExternalInput")
    x1_d = nc.dram_tensor("x1", [128, (PPC - XSPL) * KCH * B], F16,
                          kind="ExternalInput")
    br_d = nc.dram_tensor("br", [128, NBLK * COUT], F16, kind="ExternalInput")
    o_d = nc.dram_tensor("out", [128, NBLK * COUT], F16, kind="ExternalOutput")

    gmax = max(len(pl) for pl, _ in WGROUPS)
    eng = {"sync": nc.sync, "scalar": nc.scalar}
    with tile.TileContext(nc) as tc:
        with (
            tc.tile_pool(name="const", bufs=1) as cpool,
            tc.tile_pool(name="wp", bufs=WBUFS) as wpool,
            tc.tile_pool(name="op", bufs=5) as opool,
            tc.tile_pool(name="ps", bufs=3, space="PSUM") as pspool,
        ):
            x0t = cpool.tile([128, XSPL * KCH * B], F16)
            nc.scalar.dma_start(x0t[:], x0_d[:])
            x1t = cpool.tile([128, (PPC - XSPL) * KCH * B], F16)
            brt = cpool.tile([128, NBLK * COUT], F16)

            def xs_of(p, kc):
                if p < XSPL:
                    return x0t[:, (p * KCH + kc) * B: (p * KCH + kc + 1) * B]
                q = p - XSPL
                return x1t[:, (q * KCH + kc) * B: (q * KCH + kc + 1) * B]

            def mm_patch(ps1, ps2, p, half, wt, wbase, skip):
                # half 0 -> PE cols/PSUM partitions 0:64, half 1 -> 64:128
                lo, hi = 64 * half, 64 * half + 64
                for kc in range(KCH):
                    first, last = kc == 0, kc == KCH - 1
                    xs = xs_of(p, kc)
                    woff = wbase + kc * COUT
                    nc.tensor.matmul(ps1[lo:hi, :], xs,
                                     wt[:, woff: woff + 512],
                                     start=first, stop=last,
                                     skip_group_check=skip)
                    nc.tensor.matmul(ps2[lo:hi, :], xs,
                                     wt[:, woff + 512: woff + COUT],
                                     start=first, stop=last,
                                     skip_group_check=skip)

            sidx = {s: 0 for s in sizes}
            osegs = {}
            for gi, (pl, ename) in enumerate(WGROUPS):
                gs = len(pl)
                j = sidx[gs]; sidx[gs] += 1
                wt = wpool.tile([128, gmax * WB], F8, tag="w")
                eng[ename].dma_start(wt[:, : gs * WB], w_d[gs][j])
                if gi == 0:
                    # behind the first W group on each ring: the bias tile
                    # (first needed by pair-0's eviction, ~4 pairs of PSUM
                    # slack) and the second x chunk (first needed by pair 5)
                    nc.sync.dma_start(brt[:], br_d[:])
                    nc.scalar.dma_start(x1t[:], x1_d[:])

                for i in range(0, gs, 2):
                    p0 = pl[i]
                    single = p0 == PPC - 1
                    ps1 = pspool.tile([128, 512], F32, tag="ps1", bufs=4)
                    ps2 = pspool.tile([128, 256], F32, tag="ps2")
                    mm_patch(ps1, ps2, p0, 0, wt, i * WB, False)
                    if not single:
                        # skip_group_check: CoreSim's PSUM group tracker keys
                        # on the zero region without the base partition, so
                        # the 64:128 col-tile half falsely collides with the
                        # 0:64 half. HW has_written bits are per element.
                        mm_patch(ps1, ps2, pl[i + 1], 1, wt, (i + 1) * WB,
                                 True)

                    blk = p0 // 2
                    bb = brt[:, blk * COUT: (blk + 1) * COUT]
                    if single:
                        ol = opool.tile([64, COUT], F16, tag="olast")
                        nc.vector.tensor_add(ol[:, 0:512], ps1[0:64, :],
                                             bb[0:64, 0:512])
                        nc.vector.tensor_add(ol[:, 512:COUT], ps2[0:64, :],
                                             bb[0:64, 512:COUT])
                        nc.gpsimd.dma_start(
                            o_d[0:64, NPAIR * COUT: NBLK * COUT], ol[:])
                        continue
                    si = STORE_CUTS.index(blk) if blk in STORE_CUTS else -1
                    if si >= 0:
                        osegs[blk] = (opool.tile(
                            [128, (STORE_CUTS[si + 1] - blk) * COUT], F16,
                            tag="o", name=f"oseg{blk}"), STORE_CUTS[si + 1])
                    lo_ = max(c for c in STORE_CUTS if c <= blk)
                    oseg, hi_ = osegs[lo_]
                    coff = (blk - lo_) * COUT
                    nc.vector.tensor_add(oseg[:, coff: coff + 512], ps1[:],
                                         bb[:, 0:512])
                    nc.vector.tensor_add(oseg[:, coff + 512: coff + COUT],
                                         ps2[:], bb[:, 512:COUT])
                    if blk + 1 == hi_:
                        nc.gpsimd.dma_start(
                            o_d[:, lo_ * COUT: hi_ * COUT], oseg[:])
    nc.compile()
    return nc


def _prep(x, W, b):
    import ml_dtypes
    f8 = ml_dtypes.float8_e3m4
    scale = WSCALE
    wmax = float(np.abs(W).max())
    while wmax * scale > E3M4_MAX:
        scale /= 2.0
    # patch pixels, k-transposed: xp[p, k, b], k = c*256 + r*16 + s
    xp = (x.reshape(B, CIN, G, PS, G, PS)
           .transpose(2, 4, 1, 3, 5, 0)
           .reshape(P, K, B)) * (1.0 / scale)
    # resident x: [128(kpart), p, kc, b]
    xr = np.zeros((128, PPAD, KCH, B), dtype=np.float16)
    xr[:, :P] = (xp.reshape(P, KCH, 128, B)
                 .transpose(2, 0, 1, 3).astype(np.float16))
    xr = xr.reshape(128, PPAD * KCH * B)

    # weights: wr[p, kpart, kc*COUT + o] = W[p, o, kc*128 + kpart] * scale
    wr = np.zeros((PPAD, 128, WB), dtype=f8)
    wr[:P] = np.clip(
        (W.reshape(P, COUT, KCH, 128) * scale)
        .transpose(0, 3, 2, 1).reshape(P, 128, WB),
        -E3M4_MAX, E3M4_MAX).astype(f8)

    br = np.zeros((PPAD, COUT), dtype=np.float16)
    br[:P] = b.astype(np.float16)

    sizes = sorted({len(pl) for pl, _ in WGROUPS})
    in_maps = []
    for c in range(NCORES):
        base = c * PPC
        # replicated bias [128, NBLK*COUT]: block j rows 0:64 = bias[2j]
        # (same value down the batch partitions), rows 64:128 = bias[2j+1]
        brp = np.zeros((128, NBLK, COUT), dtype=np.float16)
        brp[0:64, :NPAIR] = br[base: base + PPC - 1: 2][None, :, :]
        brp[64:128, :NPAIR] = br[base + 1: base + PPC: 2][None, :, :]
        brp[0:64, NPAIR] = br[base + PPC - 1][None, :]
        m = {
            "br": np.ascontiguousarray(brp.reshape(128, NBLK * COUT)),
            "x0": np.ascontiguousarray(
                xr[:, base * KCH * B: (base + 10) * KCH * B]),
            "x1": np.ascontiguousarray(
                xr[:, (base + 10) * KCH * B: (base + PPC) * KCH * B]),
        }
        packs = {s: [] for s in sizes}
        for pl, _ in WGROUPS:
            idx = [base + p for p in pl]
            packs[len(pl)].append(
                wr[idx].transpose(1, 0, 2).reshape(128, len(pl) * WB))
        for s in sizes:
            m[f"w{s}"] = np.ascontiguousarray(np.stack(packs[s]))
        in_maps.append(m)
    return in_maps


def kernel(x, W, b):
    global LAST_RESULT
    x = np.ascontiguousarray(np.asarray(x, dtype=np.float32))
    W = np.ascontiguousarray(np.asarray(W, dtype=np.float32))
    b = np.ascontiguousarray(np.asarray(b, dtype=np.float32))
    in_maps = _prep(x, W, b)
    key = ("nc", "v7", WBUFS)
    if key not in _CACHE:
        _CACHE[key] = _build()
    res = run_bass_kernel_spmd(
        _CACHE[key], in_maps, core_ids=list(range(NCORES)),
        trace=TRACE, trace_cores=TRACE_CORES,
    )
    LAST_RESULT = res
    # assemble: core c block j rows 0:64 -> patch c*PPC+2j, rows 64:128 ->
    # patch c*PPC+2j+1; block NPAIR rows 0:64 -> patch c*PPC+24
    out = np.empty((B, PPAD, COUT), dtype=np.float32)
    for c in range(NCORES):
        ob = np.asarray(res.results[c]["out"]).reshape(128, NBLK, COUT)
        base = c * PPC
        out[:, base: base + PPC - 1: 2] = ob[0:64, :NPAIR].astype(np.float32)
        out[:, base + 1: base + PPC: 2] = ob[64:128, :NPAIR].astype(np.float32)
        out[:, base + PPC - 1] = ob[0:64, NPAIR].astype(np.float32)
    out = out[:, :P]
    return np.ascontiguousarray(out.transpose(0, 2, 1)).reshape(B, COUT, G, G)


# revision 19
# speedup vs baseline: 1.1026x; 1.1026x over previous
"""Per-patch dynamic conv (nn_DynaMicConv) as a Bass/Tile kernel on 8 TRN2 cores.

Math: for each patch p of a 14x14 grid over a 224x224 image, out[b, :, p] =
W[p] @ patch_pixels[b, p] + bias[p], i.e. 196 independent [64,768] x [768,768]
matmuls. DMA-bound: the 462 MB (f32) weight stack is read exactly once.

v2 vs the f16 baseline (101.8us):
  * W rides as fp8 E3M4 (TRN float8e3, 4 mantissa bits): half the W bytes of
    f16 at ~1.33e-2 rel err (measured host-side; gate is 2e-2). A global
    power-of-2 scale (W*128 fits in e3m4's +-15.5 range) is folded into x
    (x/128, exact in f16), so PSUM results come out in true units; x, bias,
    and the output stay f16. Per-core traffic drops 34.4MB -> 19.7MB.
  * At that traffic the PE becomes co-critical (moving operand streams 1
    col/cycle regardless of dtype), so patches are processed in PAIRS via
    column tiling: patch 2j's matmuls land in PE columns 0-63 (PSUM
    partitions 0-63), patch 2j+1's in columns 64-127, halving PSUM and
    eviction traffic per streamed column.
  * x is loaded resident up front (one DMA); W groups alternate between the
    two HWDGE rings (sync/scalar) so per-DMA completion receipts overlap;
    output stores ride SWDGE (gpsimd) off the load rings.

Measured on 8 axon TRN2 cores: 72.8us (vs 101.8us f16 baseline), rel err
1.327e-2. Post-mortems of faster-looking variants (all measured SLOWER):
stores issued mid-ring on a HWDGE ring block W loads queued behind them
(FIFO) until the store's data is computed; SWDGE serializes ~5us per
dma_start on the Q7 so per-pair stores lose outright; and the PE never
overlaps column-tiled halves (K=128 LDWEIGHTS conflicts every row group),
so the MM stream runs ~1 col/cycle and paces the kernel at ~66us + head.

Sharding: patch-parallel, P=196 padded to 200, 25 per core (12 pairs + 1).
Output DRAM per core is [128, 13*768] f16: col block j holds pair j (rows
0:64 = patch 2j, rows 64:128 = patch 2j+1); block 12 rows 0:64 = patch 24.
"""

import numpy as np

import concourse.bacc as bacc
import concourse.mybir as mybir
import concourse.tile as tile
from concourse.bass_utils import run_bass_kernel_spmd

B, CIN, IMG, PS, G = 64, 3, 224, 16, 14
P = G * G                 # 196 patches
COUT = 768
K = CIN * PS * PS         # 768 contraction
KCH = K // 128            # 6 k-chunks
NCORES = 8
PPC = (P + NCORES - 1) // NCORES   # 25 patches per core (padded)
PPAD = PPC * NCORES                # 200
NPAIR = PPC // 2                   # 12 full pairs; patch 24 runs alone
NBLK = NPAIR + 1                   # output col blocks

# patches per W DMA (even sizes so groups hold whole pairs; taper at the end
# shortens the post-last-byte compute tail). Even-indexed groups ride the
# sync (SP) HWDGE ring, odd-indexed the scalar (ACT) ring.
GROUPS = [4, 6, 6, 6, 2, 1]
assert sum(GROUPS) == PPC
# output store cuts, in pair blocks (final single patch stored separately)
STORE_CUTS = [0, 2, 5, 8, 11, 12]

F32 = mybir.dt.float32
F16 = mybir.dt.float16
F8 = mybir.dt.float8e3   # TRN E3M4

WSCALE = 128.0           # power of 2: folded into x exactly
E3M4_MAX = 15.5

WBUFS = 3

# test.py hooks
TRACE = False
TRACE_CORES = [0]
LAST_RESULT = None

_CACHE = {}

WB = KCH * COUT          # W bytes per patch per partition (fp8)


def _build():
    nc = bacc.Bacc("TRN2", target_bir_lowering=False, debug=False)
    sizes = sorted(set(GROUPS))
    cnt = {s: GROUPS.count(s) for s in sizes}
    w_d = {s: nc.dram_tensor(f"w{s}", [cnt[s], 128, s * WB], F8,
                             kind="ExternalInput") for s in sizes}
    x_d = nc.dram_tensor("x", [128, PPC * KCH * B], F16, kind="ExternalInput")
    bo_d = nc.dram_tensor("bo", [1, B + PPC * COUT], F16, kind="ExternalInput")
    o_d = nc.dram_tensor("out", [128, NBLK * COUT], F16, kind="ExternalOutput")

    gmax = max(GROUPS)
    with tile.TileContext(nc) as tc:
        with (
            tc.tile_pool(name="const", bufs=1) as cpool,
            tc.tile_pool(name="wp", bufs=WBUFS) as wpool,
            tc.tile_pool(name="op", bufs=3) as opool,
            tc.tile_pool(name="ps", bufs=3, space="PSUM") as pspool,
        ):
            bo = cpool.tile([1, B + PPC * COUT], F16)
            nc.scalar.dma_start(bo[:], bo_d[:])
            xt = cpool.tile([128, PPC * KCH * B], F16)
            nc.scalar.dma_start(xt[:], x_d[:])
            ones = bo[:, 0:B]

            def bias(p):  # [1, 768] slice for patch p
                return bo[:, B + p * COUT: B + (p + 1) * COUT]

            sidx = {s: 0 for s in sizes}
            poff = 0          # first patch of current group
            seg = 0           # store segment index (in pair blocks)
            oseg = None
            for gi, GPS in enumerate(GROUPS):
                j = sidx[GPS]; sidx[GPS] += 1
                wt = wpool.tile([128, gmax * WB], F8, tag="w")
                eng = nc.sync if gi % 2 == 0 else nc.scalar
                eng.dma_start(wt[:, : GPS * WB], w_d[GPS][j])

                for i in range(0, GPS, 2):
                    p0 = poff + i
                    single = p0 == PPC - 1
                    ps1 = pspool.tile([128, 512], F32, tag="ps1", bufs=4)
                    ps2 = pspool.tile([128, 256], F32, tag="ps2")
                    b0 = bias(p0)
                    nc.tensor.matmul(ps1[0:64, :], ones, b0[:, 0:512],
                                     start=True, stop=False)
                    nc.tensor.matmul(ps2[0:64, :], ones, b0[:, 512:COUT],
                                     start=True, stop=False)
                    if not single:
                        # skip_group_check: CoreSim's PSUM accumulation-group
                        # tracker keys on the zero region without the base
                        # partition, so the 64:128 col-tile half falsely
                        # collides with the 0:64 half. HW has_written bits
                        # are per element; the halves are disjoint.
                        b1 = bias(p0 + 1)
                        nc.tensor.matmul(ps1[64:128, :], ones, b1[:, 0:512],
                                         start=True, stop=False,
                                         skip_group_check=True)
                        nc.tensor.matmul(ps2[64:128, :], ones, b1[:, 512:COUT],
                                         start=True, stop=False,
                                         skip_group_check=True)
                    for kc in range(KCH):
                        last = kc == KCH - 1
                        for h in range(1 if single else 2):
                            p = p0 + h
                            lo, hi = 64 * h, 64 * h + 64
                            xs = xt[:, (p * KCH + kc) * B: (p * KCH + kc + 1) * B]
                            woff = ((i + h) * KCH + kc) * COUT
                            nc.tensor.matmul(ps1[lo:hi, :], xs,
                                             wt[:, woff: woff + 512],
                                             start=False, stop=last,
                                             skip_group_check=h == 1)
                            nc.tensor.matmul(ps2[lo:hi, :], xs,
                                             wt[:, woff + 512: woff + COUT],
                                             start=False, stop=last,
                                             skip_group_check=h == 1)

                    blk = p0 // 2
                    if single:
                        oseg = opool.tile([64, COUT], F16, tag="olast")
                        nc.vector.tensor_copy(oseg[:, 0:512], ps1[0:64, :])
                        nc.vector.tensor_copy(oseg[:, 512:COUT], ps2[0:64, :])
                        nc.gpsimd.dma_start(
                            o_d[0:64, NPAIR * COUT: NBLK * COUT], oseg[:])
                        continue
                    if blk == STORE_CUTS[seg]:
                        nseg = STORE_CUTS[seg + 1] - STORE_CUTS[seg]
                        oseg = opool.tile([128, nseg * COUT], F16, tag="o",
                                          name=f"oseg{seg}")
                    coff = (blk - STORE_CUTS[seg]) * COUT
                    nc.vector.tensor_copy(oseg[:, coff: coff + 512], ps1[:])
                    nc.vector.tensor_copy(oseg[:, coff + 512: coff + COUT],
                                          ps2[:])
                    if blk + 1 == STORE_CUTS[seg + 1]:
                        nc.gpsimd.dma_start(
                            o_d[:, STORE_CUTS[seg] * COUT:
                                 STORE_CUTS[seg + 1] * COUT], oseg[:])
                        seg += 1
                poff += GPS
    nc.compile()
    return nc


def _prep(x, W, b):
    import ml_dtypes
    f8 = ml_dtypes.float8_e3m4
    scale = WSCALE
    wmax = float(np.abs(W).max())
    while wmax * scale > E3M4_MAX:
        scale /= 2.0
    # patch pixels, k-transposed: xp[p, k, b], k = c*256 + r*16 + s
    xp = (x.reshape(B, CIN, G, PS, G, PS)
           .transpose(2, 4, 1, 3, 5, 0)
           .reshape(P, K, B)) * (1.0 / scale)
    # resident x: [128(kpart), p, kc, b]
    xr = np.zeros((128, PPAD, KCH, B), dtype=np.float16)
    xr[:, :P] = (xp.reshape(P, KCH, 128, B)
                 .transpose(2, 0, 1, 3).astype(np.float16))
    xr = xr.reshape(128, PPAD * KCH * B)

    # weights: wr[p, kpart, kc*COUT + o] = W[p, o, kc*128 + kpart] * scale
    wr = np.zeros((PPAD, 128, WB), dtype=f8)
    wr[:P] = np.clip(
        (W.reshape(P, COUT, KCH, 128) * scale)
        .transpose(0, 3, 2, 1).reshape(P, 128, WB),
        -E3M4_MAX, E3M4_MAX).astype(f8)

    br = np.zeros((PPAD, COUT), dtype=np.float16)
    br[:P] = b.astype(np.float16)

    sizes = sorted(set(GROUPS))
    in_maps = []
    for c in range(NCORES):
        base = c * PPC
        bo = np.empty((1, B + PPC * COUT), dtype=np.float16)
        bo[0, :B] = 1.0
        bo[0, B:] = br[base: base + PPC].reshape(-1)
        m = {
            "bo": bo,
            "x": np.ascontiguousarray(
                xr[:, base * KCH * B: (base + PPC) * KCH * B]),
        }
        packs = {s: [] for s in sizes}
        poff = 0
        for gs in GROUPS:
            pl = slice(base + poff, base + poff + gs)
            packs[gs].append(
                wr[pl].transpose(1, 0, 2).reshape(128, gs * WB))
            poff += gs
        for s in sizes:
            m[f"w{s}"] = np.ascontiguousarray(np.stack(packs[s]))
        in_maps.append(m)
    return in_maps


def kernel(x, W, b):
    global LAST_RESULT
    x = np.ascontiguousarray(np.asarray(x, dtype=np.float32))
    W = np.ascontiguousarray(np.asarray(W, dtype=np.float32))
    b = np.ascontiguousarray(np.asarray(b, dtype=np.float32))
    in_maps = _prep(x, W, b)
    key = ("nc", "v2", WBUFS)
    if key not in _CACHE:
        _CACHE[key] = _build()
    res = run_bass_kernel_spmd(
        _CACHE[key], in_maps, core_ids=list(range(NCORES)),
        trace=TRACE, trace_cores=TRACE_CORES,
    )
    LAST_RESULT = res
    # assemble: core c block j rows 0:64 -> patch c*PPC+2j, rows 64:128 ->
    # patch c*PPC+2j+1; block NPAIR rows 0:64 -> patch c*PPC+24
    out = np.empty((B, PPAD, COUT), dtype=np.float32)
    for c in range(NCORES):
        ob = np.asarray(res.results[c]["out"]).reshape(128, NBLK, COUT)
        base = c * PPC
        out[:, base: base + PPC - 1: 2] = ob[0:64, :NPAIR].astype(np.float32)
        out[:, base + 1: base + PPC: 2] = ob[64:128, :NPAIR].astype(np.float32)
        out[:, base + PPC - 1] = ob[0:64, NPAIR].astype(np.float32)
    out = out[:, :P]
    return np.ascontiguousarray(out.transpose(0, 2, 1)).reshape(B, COUT, G, G)
